# revision 2
# baseline (speedup 1.0000x reference)
"""Multi-head causal attention on 8 trn2 NeuronCores.

Sharding: data-parallel over batch (2) x tensor-parallel over heads (4 per
core, Megatron-style column-split QKV / row-split output projection).
Per-core partial outputs are summed on the host (+ output bias).
"""

import sys

sys.path.insert(0, "/opt/trn_rl_repo")

import ml_dtypes
import numpy as np

import concourse.bass as bass  # noqa: F401  (import keeps bass registered)
import concourse.tile as tile
from concourse import bacc, mybir

BF16 = mybir.dt.bfloat16
F32 = mybir.dt.float32
AF = mybir.ActivationFunctionType

N = 2048  # sequence length
D = 1024  # model dim
NC = 8  # cores


def build_nc():
    """Build the (SPMD) Bass program run identically on all 8 cores."""
    nc = bacc.Bacc("TRN2", target_bir_lowering=False, debug=False, num_devices=NC)

    xT = nc.declare_dram_parameter("xT", [8, 128, N], BF16, isOutput=False)
    wq = nc.declare_dram_parameter("wq", [8, 128, 256], BF16, isOutput=False)
    wk = nc.declare_dram_parameter("wk", [8, 128, 256], BF16, isOutput=False)
    wv = nc.declare_dram_parameter("wv", [8, 128, 260], BF16, isOutput=False)
    bqp = nc.declare_dram_parameter("bq", [128, 2], F32, isOutput=False)
    bkp = nc.declare_dram_parameter("bk", [128, 2], F32, isOutput=False)
    bvcp = nc.declare_dram_parameter("bvc", [1, 260], BF16, isOutput=False)
    wo = nc.declare_dram_parameter("wo", [128, 2, 1024], BF16, isOutput=False)
    maskp = nc.declare_dram_parameter("mask", [128, 128], BF16, isOutput=False)
    outp = nc.declare_dram_parameter("out", [N, 1024], F32, isOutput=True)

    with tile.TileContext(nc) as tc:
        with tc.tile_pool(name="singles", bufs=1) as singles:
            xt_sb = singles.tile([128, 8, N], BF16)
            wq_sb = singles.tile([128, 8, 256], BF16)
            wk_sb = singles.tile([128, 8, 256], BF16)
            wv_sb = singles.tile([128, 8, 260], BF16)
            bq_sb = singles.tile([128, 2], F32)
            bk_sb = singles.tile([128, 2], F32)
            bvc_sb = singles.tile([1, 260], BF16)
            wo_sb = singles.tile([128, 2, 1024], BF16)
            mask_sb = singles.tile([128, 128], BF16)
            ones_sb = singles.tile([1, 128], BF16)
            qT_sb = singles.tile([128, 2, N], BF16)
            kT_sb = singles.tile([128, 2, N], BF16)
            vc_sb = singles.tile([128, 16, 260], BF16)
            ctxn_sb = singles.tile([128, 2, N], BF16)

            nc.vector.memset(ones_sb[:, :], 1.0)
            for kc in range(8):
                nc.sync.dma_start(out=xt_sb[:, kc, :], in_=xT[kc])
                nc.sync.dma_start(out=wq_sb[:, kc, :], in_=wq[kc])
                nc.sync.dma_start(out=wk_sb[:, kc, :], in_=wk[kc])
                nc.sync.dma_start(out=wv_sb[:, kc, :], in_=wv[kc])
            nc.sync.dma_start(out=bq_sb[:, :], in_=bqp[:, :])
            nc.sync.dma_start(out=bk_sb[:, :], in_=bkp[:, :])
            nc.sync.dma_start(out=bvc_sb[:, :], in_=bvcp[:, :])
            nc.sync.dma_start(out=wo_sb[:, :, :], in_=wo[:, :, :])
            nc.sync.dma_start(out=mask_sb[:, :], in_=maskp[:, :])

            # ---------------- projections: qT/kT [d, i] and v [j, d|1] ----
            with tc.tile_pool(name="qk_ps", bufs=2, space="PSUM") as qk_ps, \
                 tc.tile_pool(name="v_ps", bufs=2, space="PSUM") as v_ps:
                for w_sb, b_sb, o_sb in (
                    (wq_sb, bq_sb, qT_sb),
                    (wk_sb, bk_sb, kT_sb),
                ):
                    for c in range(2):
                        for I in range(4):
                            ps = qk_ps.tile([128, 512], F32, tag="qk")
                            for kc in range(8):
                                nc.tensor.matmul(
                                    ps[:, :],
                                    lhsT=w_sb[:, kc, 128 * c : 128 * (c + 1)],
                                    rhs=xt_sb[:, kc, 512 * I : 512 * (I + 1)],
                                    start=(kc == 0),
                                    stop=(kc == 7),
                                )
                            nc.scalar.activation(
                                o_sb[:, c, 512 * I : 512 * (I + 1)],
                                ps[:, :],
                                AF.Identity,
                                bias=b_sb[:, c : c + 1],
                            )
                for J in range(16):
                    ps = v_ps.tile([128, 260], F32, tag="v")
                    for kc in range(8):
                        nc.tensor.matmul(
                            ps[:, :],
                            lhsT=xt_sb[:, kc, 128 * J : 128 * (J + 1)],
                            rhs=wv_sb[:, kc, :],
                            start=(kc == 0),
                            stop=False,
                        )
                    nc.tensor.matmul(
                        ps[:, :],
                        lhsT=ones_sb[:, :],
                        rhs=bvc_sb[:, :],
                        start=False,
                        stop=True,
                    )
                    nc.vector.tensor_copy(out=vc_sb[:, J, :], in_=ps[:, :])

            # ---------------- attention (sT[j, i] orientation) ------------
            with tc.tile_pool(name="sc_ps", bufs=2, space="PSUM") as scp, \
                 tc.tile_pool(name="ctx_ps", bufs=1, space="PSUM") as ctxp, \
                 tc.tile_pool(name="pt", bufs=3) as ptp, \
                 tc.tile_pool(name="zn", bufs=2) as znp:
                for h in range(4):
                    c, po = h // 2, 64 * (h % 2)
                    ctx_t = [
                        ctxp.tile([65, 512], F32, name=f"ctx{I}", tag=f"ctx{I}")
                        for I in range(4)
                    ]
                    for J in range(16):
                        L = N - 128 * J  # valid i range starts at the diagonal
                        pt = ptp.tile([128, N], BF16, tag="pt")
                        off = 0
                        while off < L:
                            w = min(1024, L - off)
                            ps = scp.tile([128, 1024], F32, tag="sc")
                            for s in range(0, w, 512):
                                sw = min(512, w - s)
                                gi = 128 * J + off + s
                                nc.tensor.matmul(
                                    ps[:, s : s + sw],
                                    lhsT=kT_sb[po : po + 64, c, 128 * J : 128 * (J + 1)],
                                    rhs=qT_sb[po : po + 64, c, gi : gi + sw],
                                    start=True,
                                    stop=True,
                                )
                            nc.scalar.activation(
                                pt[:, off : off + w],
                                ps[:, :w],
                                AF.Exp,
                                scale=0.125,
                            )
                            off += w
                        # zero the upper-triangular (j > i) part of the
                        # diagonal block (first 128 columns)
                        nc.vector.tensor_mul(
                            pt[:, :128], pt[:, :128], mask_sb[:, :]
                        )
                        for I in range(J // 4, 4):
                            gs = max(512 * I, 128 * J)
                            ge = 512 * (I + 1)
                            nc.tensor.matmul(
                                ctx_t[I][:, gs - 512 * I : ge - 512 * I],
                                lhsT=vc_sb[:, J, 65 * h : 65 * h + 65],
                                rhs=pt[:, gs - 128 * J : ge - 128 * J],
                                start=(J == 0),
                                stop=(J == 4 * I + 3),
                            )
                    for I in range(4):
                        zr = znp.tile([1, 512], F32, tag="zr")
                        nc.vector.reciprocal(zr[:, :], ctx_t[I][64:65, :])
                        zb = znp.tile([64, 512], F32, tag="zb")
                        nc.gpsimd.partition_broadcast(zb[:, :], zr[:, :], channels=64)
                        nc.vector.tensor_mul(
                            ctxn_sb[po : po + 64, c, 512 * I : 512 * (I + 1)],
                            ctx_t[I][0:64, :],
                            zb[:, :],
                        )

            # ---------------- output projection (row-parallel) ------------
            with tc.tile_pool(name="f_ps", bufs=4, space="PSUM") as fp, \
                 tc.tile_pool(name="osb", bufs=4) as osb:
                for t in range(16):
                    for oc in range(2):
                        ps = fp.tile([128, 512], F32, tag="f")
                        for a in range(2):
                            nc.tensor.matmul(
                                ps[:, :],
                                lhsT=ctxn_sb[:, a, 128 * t : 128 * (t + 1)],
                                rhs=wo_sb[:, a, 512 * oc : 512 * (oc + 1)],
                                start=(a == 0),
                                stop=(a == 1),
                            )
                        ot = osb.tile([128, 512], F32, tag="o")
                        nc.vector.tensor_copy(out=ot[:, :], in_=ps[:, :])
                        nc.sync.dma_start(
                            out=outp[128 * t : 128 * (t + 1), 512 * oc : 512 * (oc + 1)],
                            in_=ot[:, :],
                        )

    nc.compile()
    return nc


class _Runner:
    """Jitted PJRT executor for the SPMD program (built once per process)."""

    def __init__(self, nc):
        import jax
        from jax.experimental.shard_map import shard_map
        from jax.sharding import Mesh, NamedSharding, PartitionSpec

        from concourse.bass2jax import (
            _bass_exec_p,
            install_neuronx_cc_hook,
            partition_id_tensor,
        )

        install_neuronx_cc_hook()
        self.nc = nc
        self.jax = jax

        in_names, out_names, out_avals = [], [], []
        partition_name = (
            nc.partition_id_tensor.name if nc.partition_id_tensor else None
        )
        for alloc in nc.m.functions[0].allocations:
            if not isinstance(alloc, mybir.MemoryLocationSet):
                continue
            name = alloc.memorylocations[0].name
            if alloc.kind == "ExternalInput":
                if name != partition_name:
                    in_names.append(name)
            elif alloc.kind == "ExternalOutput":
                out_names.append(name)
                out_avals.append(
                    jax.core.ShapedArray(
                        tuple(alloc.tensor_shape), mybir.dt.np(alloc.dtype)
                    )
                )
        self.in_names = list(in_names)
        self.out_names = out_names
        self.out_avals = out_avals
        n_params = len(in_names)
        n_outs = len(out_names)
        all_names = in_names + out_names
        if partition_name is not None:
            all_names = all_names + [partition_name]

        def _body(*args):
            operands = list(args)
            if partition_name is not None:
                operands.append(partition_id_tensor())
            return tuple(
                _bass_exec_p.bind(
                    *operands,
                    out_avals=tuple(out_avals),
                    in_names=tuple(all_names),
                    out_names=tuple(out_names),
                    lowering_input_output_aliases=(),
                    sim_require_finite=True,
                    sim_require_nnan=True,
                    nc=nc,
                )
            )

        devices = jax.devices()[:NC]
        self.mesh = Mesh(np.asarray(devices), ("core",))
        in_specs = (PartitionSpec("core"),) * (n_params + n_outs)
        out_specs = (PartitionSpec("core"),) * n_outs
        self.fn = jax.jit(
            shard_map(
                _body,
                mesh=self.mesh,
                in_specs=in_specs,
                out_specs=out_specs,
                check_rep=False,
            ),
            keep_unused=True,
        )
        self.sharding = NamedSharding(self.mesh, PartitionSpec("core"))

    def prep(self, in_maps):
        """Concatenate per-core inputs along axis 0 and device_put."""
        arrs = []
        for i, name in enumerate(self.in_names):
            arrs.append(np.concatenate([m[name] for m in in_maps], axis=0))
        for av in self.out_avals:
            arrs.append(np.zeros((NC * av.shape[0], *av.shape[1:]), av.dtype))
        return [self.jax.device_put(a, self.sharding) for a in arrs]

    def run(self, dev_args):
        out = self.fn(*dev_args)
        self.jax.block_until_ready(out)
        return out

    def unpack(self, out):
        res = []
        for c in range(NC):
            res.append(
                {
                    name: np.asarray(out[i]).reshape(NC, *self.out_avals[i].shape)[c]
                    for i, name in enumerate(self.out_names)
                }
            )
        return res


_RUNNER = None


def _get_runner():
    global _RUNNER
    if _RUNNER is None:
        _RUNNER = _Runner(build_nc())
    return _RUNNER


def make_in_maps(x, Wq, bq, Wk, bk, Wv, bv, Wo, bo):
    bf = ml_dtypes.bfloat16
    f32 = np.float32
    x = np.asarray(x, f32)
    mask = np.ascontiguousarray(np.triu(np.ones((128, 128), f32))).astype(bf)
    in_maps = []
    for core in range(NC):
        b, g = core // 4, core % 4
        sl = slice(256 * g, 256 * (g + 1))
        wv_cat = np.zeros((D, 260), f32)
        bv_cat = np.zeros((1, 260), f32)
        for h in range(4):
            col = 256 * g + 64 * h
            wv_cat[:, 65 * h : 65 * h + 64] = Wv[:, col : col + 64]
            bv_cat[0, 65 * h : 65 * h + 64] = bv[col : col + 64]
            bv_cat[0, 65 * h + 64] = 1.0
        in_maps.append(
            {
                "xT": np.ascontiguousarray(x[b].T).reshape(8, 128, N).astype(bf),
                "wq": np.ascontiguousarray(Wq[:, sl]).reshape(8, 128, 256).astype(bf),
                "wk": np.ascontiguousarray(Wk[:, sl]).reshape(8, 128, 256).astype(bf),
                "wv": wv_cat.reshape(8, 128, 260).astype(bf),
                "bq": np.ascontiguousarray(np.asarray(bq, f32)[sl].reshape(2, 128).T),
                "bk": np.ascontiguousarray(np.asarray(bk, f32)[sl].reshape(2, 128).T),
                "bvc": bv_cat.astype(bf),
                "wo": np.ascontiguousarray(
                    np.asarray(Wo, f32)[sl].reshape(2, 128, 1024).transpose(1, 0, 2)
                ).astype(bf),
                "mask": mask,
            }
        )
    return in_maps


def combine(results, bo):
    out = np.zeros((2, N, D), np.float32)
    for core in range(NC):
        out[core // 4] += results[core]["out"]
    out += np.asarray(bo, np.float32)[None, None, :]
    return out


def kernel(x, Wq, bq, Wk, bk, Wv, bv, Wo, bo):
    runner = _get_runner()
    in_maps = make_in_maps(x, Wq, bq, Wk, bk, Wv, bv, Wo, bo)
    dev_args = runner.prep(in_maps)
    results = runner.unpack(runner.run(dev_args))
    return combine(results, bo)


# revision 6
# speedup vs baseline: 294.3577x; 294.3577x over previous
"""Multi-head causal attention on 8 trn2 NeuronCores.

Sharding: data-parallel over batch (2) x tensor-parallel over heads (4 per
core, Megatron-style column-split QKV / row-split output projection).
Per-core partial outputs are summed on the host (+ output bias).
"""

import sys

sys.path.insert(0, "/opt/trn_rl_repo")

import ml_dtypes
import numpy as np

import concourse.bass as bass  # noqa: F401  (import keeps bass registered)
import concourse.tile as tile
from concourse import bacc, mybir

BF16 = mybir.dt.bfloat16
F32 = mybir.dt.float32
AF = mybir.ActivationFunctionType

N = 2048  # sequence length
D = 1024  # model dim
NC = 8  # cores


def build_nc(variant="full", loop=1):
    """Build the (SPMD) Bass program run identically on all 8 cores.

    variant: "full" | "nopb" (skip partition_broadcast, copy unnormalized ctx)
    loop: repeat the whole body N times inside the NEFF (timing harness).
    """
    nc = bacc.Bacc("TRN2", target_bir_lowering=False, debug=False, num_devices=NC)

    xT = nc.declare_dram_parameter("xT", [8, 128, N], BF16, isOutput=False)
    wq = nc.declare_dram_parameter("wq", [8, 128, 256], BF16, isOutput=False)
    wk = nc.declare_dram_parameter("wk", [8, 128, 256], BF16, isOutput=False)
    wv = nc.declare_dram_parameter("wv", [8, 128, 260], BF16, isOutput=False)
    bqp = nc.declare_dram_parameter("bq", [128, 2], F32, isOutput=False)
    bkp = nc.declare_dram_parameter("bk", [128, 2], F32, isOutput=False)
    bvcp = nc.declare_dram_parameter("bvc", [1, 260], BF16, isOutput=False)
    wo = nc.declare_dram_parameter("wo", [128, 2, 1024], BF16, isOutput=False)
    maskp = nc.declare_dram_parameter("mask", [128, 128], BF16, isOutput=False)
    outp = nc.declare_dram_parameter("out", [N, 1024], F32, isOutput=True)

    with tile.TileContext(nc) as tc:
        with tc.tile_pool(name="singles", bufs=1) as singles:
            xt_sb = singles.tile([128, 8, N], BF16)
            wq_sb = singles.tile([128, 8, 256], BF16)
            wk_sb = singles.tile([128, 8, 256], BF16)
            wv_sb = singles.tile([128, 8, 260], BF16)
            bq_sb = singles.tile([128, 2], F32)
            bk_sb = singles.tile([128, 2], F32)
            bvc_sb = singles.tile([1, 260], BF16)
            wo_sb = singles.tile([128, 2, 1024], BF16)
            mask_sb = singles.tile([128, 128], BF16)
            ones_sb = singles.tile([1, 128], BF16)
            qT_sb = singles.tile([128, 2, N], BF16)
            kT_sb = singles.tile([128, 2, N], BF16)
            vc_sb = singles.tile([128, 16, 260], BF16)
            ctxn_sb = singles.tile([128, 2, N], BF16)

            def _dma_in():
                nc.vector.memset(ones_sb[:, :], 1.0)
                for kc in range(8):
                    nc.sync.dma_start(out=xt_sb[:, kc, :], in_=xT[kc])
                    nc.sync.dma_start(out=wq_sb[:, kc, :], in_=wq[kc])
                    nc.sync.dma_start(out=wk_sb[:, kc, :], in_=wk[kc])
                    nc.sync.dma_start(out=wv_sb[:, kc, :], in_=wv[kc])
                nc.sync.dma_start(out=bq_sb[:, :], in_=bqp[:, :])
                nc.sync.dma_start(out=bk_sb[:, :], in_=bkp[:, :])
                nc.sync.dma_start(out=bvc_sb[:, :], in_=bvcp[:, :])
                nc.sync.dma_start(out=wo_sb[:, :, :], in_=wo[:, :, :])
                nc.sync.dma_start(out=mask_sb[:, :], in_=maskp[:, :])

            def _projections():
                with tc.tile_pool(name="qk_ps", bufs=2, space="PSUM") as qk_ps, \
                     tc.tile_pool(name="v_ps", bufs=2, space="PSUM") as v_ps:
                    for w_sb, b_sb, o_sb in (
                        (wq_sb, bq_sb, qT_sb),
                        (wk_sb, bk_sb, kT_sb),
                    ):
                        for c in range(2):
                            for I in range(4):
                                ps = qk_ps.tile([128, 512], F32, tag="qk")
                                for kc in range(8):
                                    nc.tensor.matmul(
                                        ps[:, :],
                                        lhsT=w_sb[:, kc, 128 * c : 128 * (c + 1)],
                                        rhs=xt_sb[:, kc, 512 * I : 512 * (I + 1)],
                                        start=(kc == 0),
                                        stop=(kc == 7),
                                    )
                                nc.scalar.activation(
                                    o_sb[:, c, 512 * I : 512 * (I + 1)],
                                    ps[:, :],
                                    AF.Identity,
                                    bias=b_sb[:, c : c + 1],
                                )
                    for J in range(16):
                        ps = v_ps.tile([128, 260], F32, tag="v")
                        for kc in range(8):
                            nc.tensor.matmul(
                                ps[:, :],
                                lhsT=xt_sb[:, kc, 128 * J : 128 * (J + 1)],
                                rhs=wv_sb[:, kc, :],
                                start=(kc == 0),
                                stop=False,
                            )
                        nc.tensor.matmul(
                            ps[:, :],
                            lhsT=ones_sb[:, :],
                            rhs=bvc_sb[:, :],
                            start=False,
                            stop=True,
                        )
                        nc.vector.tensor_copy(out=vc_sb[:, J, :], in_=ps[:, :])

            def _attention():
                with tc.tile_pool(name="sc_ps", bufs=2, space="PSUM") as scp, \
                     tc.tile_pool(name="ctx_ps", bufs=1, space="PSUM") as ctxp, \
                     tc.tile_pool(name="pt", bufs=3) as ptp, \
                     tc.tile_pool(name="zn", bufs=2) as znp:
                    for h in range(4):
                        c, po = h // 2, 64 * (h % 2)
                        ctx_t = [
                            ctxp.tile([65, 512], F32, name=f"ctx{I}", tag=f"ctx{I}")
                            for I in range(4)
                        ]
                        for J in range(16):
                            L = N - 128 * J  # valid i starts at the diagonal
                            pt = ptp.tile([128, N], BF16, tag="pt")
                            off = 0
                            while off < L:
                                w = min(1024, L - off)
                                ps = scp.tile([128, 1024], F32, tag="sc")
                                for s in range(0, w, 512):
                                    sw = min(512, w - s)
                                    gi = 128 * J + off + s
                                    nc.tensor.matmul(
                                        ps[:, s : s + sw],
                                        lhsT=kT_sb[
                                            po : po + 64, c, 128 * J : 128 * (J + 1)
                                        ],
                                        rhs=qT_sb[po : po + 64, c, gi : gi + sw],
                                        start=True,
                                        stop=True,
                                    )
                                nc.scalar.activation(
                                    pt[:, off : off + w],
                                    ps[:, :w],
                                    AF.Exp,
                                    scale=0.125,
                                )
                                off += w
                            # zero the j > i part of the diagonal block
                            nc.vector.tensor_mul(
                                pt[:, :128], pt[:, :128], mask_sb[:, :]
                            )
                            for I in range(J // 4, 4):
                                gs = max(512 * I, 128 * J)
                                ge = 512 * (I + 1)
                                nc.tensor.matmul(
                                    ctx_t[I][:, gs - 512 * I : ge - 512 * I],
                                    lhsT=vc_sb[:, J, 65 * h : 65 * h + 65],
                                    rhs=pt[:, gs - 128 * J : ge - 128 * J],
                                    start=(J == 0),
                                    stop=(J == 4 * I + 3),
                                )
                        for I in range(4):
                            if variant == "nopb":
                                nc.vector.tensor_copy(
                                    out=ctxn_sb[
                                        po : po + 64, c, 512 * I : 512 * (I + 1)
                                    ],
                                    in_=ctx_t[I][0:64, :],
                                )
                                continue
                            zr = znp.tile([1, 512], F32, tag="zr")
                            nc.vector.reciprocal(zr[:, :], ctx_t[I][64:65, :])
                            zb = znp.tile([64, 512], F32, tag="zb")
                            nc.gpsimd.partition_broadcast(
                                zb[:, :], zr[:, :], channels=64
                            )
                            nc.vector.tensor_mul(
                                ctxn_sb[po : po + 64, c, 512 * I : 512 * (I + 1)],
                                ctx_t[I][0:64, :],
                                zb[:, :],
                            )

            def _final():
                with tc.tile_pool(name="f_ps", bufs=4, space="PSUM") as fp, \
                     tc.tile_pool(name="osb", bufs=4) as osb:
                    for t in range(16):
                        for oc in range(2):
                            ps = fp.tile([128, 512], F32, tag="f")
                            for a in range(2):
                                nc.tensor.matmul(
                                    ps[:, :],
                                    lhsT=ctxn_sb[:, a, 128 * t : 128 * (t + 1)],
                                    rhs=wo_sb[:, a, 512 * oc : 512 * (oc + 1)],
                                    start=(a == 0),
                                    stop=(a == 1),
                                )
                            ot = osb.tile([128, 512], F32, tag="o")
                            nc.vector.tensor_copy(out=ot[:, :], in_=ps[:, :])
                            nc.sync.dma_start(
                                out=outp[
                                    128 * t : 128 * (t + 1),
                                    512 * oc : 512 * (oc + 1),
                                ],
                                in_=ot[:, :],
                            )

            def _iter():
                _dma_in()
                _projections()
                _attention()
                _final()

            if loop == 1:
                _iter()
            else:
                with tc.For_i(0, loop, 1):
                    _iter()

    nc.compile()
    return nc


class _Runner:
    """Jitted PJRT executor for the SPMD program (built once per process)."""

    def __init__(self, nc):
        import jax
        from jax.experimental.shard_map import shard_map
        from jax.sharding import Mesh, NamedSharding, PartitionSpec

        from concourse.bass2jax import (
            _bass_exec_p,
            install_neuronx_cc_hook,
            partition_id_tensor,
        )

        install_neuronx_cc_hook()
        self.nc = nc
        self.jax = jax

        in_names, out_names, out_avals = [], [], []
        partition_name = (
            nc.partition_id_tensor.name if nc.partition_id_tensor else None
        )
        for alloc in nc.m.functions[0].allocations:
            if not isinstance(alloc, mybir.MemoryLocationSet):
                continue
            name = alloc.memorylocations[0].name
            if alloc.kind == "ExternalInput":
                if name != partition_name:
                    in_names.append(name)
            elif alloc.kind == "ExternalOutput":
                out_names.append(name)
                out_avals.append(
                    jax.core.ShapedArray(
                        tuple(alloc.tensor_shape), mybir.dt.np(alloc.dtype)
                    )
                )
        self.in_names = list(in_names)
        self.out_names = out_names
        self.out_avals = out_avals
        n_params = len(in_names)
        n_outs = len(out_names)
        all_names = in_names + out_names
        if partition_name is not None:
            all_names = all_names + [partition_name]

        def _body(*args):
            operands = list(args)
            if partition_name is not None:
                operands.append(partition_id_tensor())
            return tuple(
                _bass_exec_p.bind(
                    *operands,
                    out_avals=tuple(out_avals),
                    in_names=tuple(all_names),
                    out_names=tuple(out_names),
                    lowering_input_output_aliases=(),
                    sim_require_finite=True,
                    sim_require_nnan=True,
                    nc=nc,
                )
            )

        devices = jax.devices()[:NC]
        self.mesh = Mesh(np.asarray(devices), ("core",))
        in_specs = (PartitionSpec("core"),) * (n_params + n_outs)
        out_specs = (PartitionSpec("core"),) * n_outs
        self.fn = jax.jit(
            shard_map(
                _body,
                mesh=self.mesh,
                in_specs=in_specs,
                out_specs=out_specs,
                check_rep=False,
            ),
            keep_unused=True,
        )
        self.sharding = NamedSharding(self.mesh, PartitionSpec("core"))

    def prep(self, in_maps):
        """Concatenate per-core inputs along axis 0 and device_put."""
        arrs = []
        for name in self.in_names:
            arrs.append(np.concatenate([m[name] for m in in_maps], axis=0))
        for av in self.out_avals:
            arrs.append(np.zeros((NC * av.shape[0], *av.shape[1:]), av.dtype))
        return [self.jax.device_put(a, self.sharding) for a in arrs]

    def run(self, dev_args):
        out = self.fn(*dev_args)
        self.jax.block_until_ready(out)
        return out

    def run_async(self, dev_args):
        return self.fn(*dev_args)

    def unpack(self, out):
        res = []
        for c in range(NC):
            res.append(
                {
                    name: np.asarray(out[i]).reshape(NC, *self.out_avals[i].shape)[c]
                    for i, name in enumerate(self.out_names)
                }
            )
        return res


_RUNNER = None


def _get_runner():
    global _RUNNER
    if _RUNNER is None:
        _RUNNER = _Runner(build_nc())
    return _RUNNER


def make_in_maps(x, Wq, bq, Wk, bk, Wv, bv, Wo, bo):
    bf = ml_dtypes.bfloat16
    f32 = np.float32
    x = np.asarray(x, f32)
    mask = np.ascontiguousarray(np.triu(np.ones((128, 128), f32))).astype(bf)
    in_maps = []
    for core in range(NC):
        b, g = core // 4, core % 4
        sl = slice(256 * g, 256 * (g + 1))
        wv_cat = np.zeros((D, 260), f32)
        bv_cat = np.zeros((1, 260), f32)
        for h in range(4):
            col = 256 * g + 64 * h
            wv_cat[:, 65 * h : 65 * h + 64] = Wv[:, col : col + 64]
            bv_cat[0, 65 * h : 65 * h + 64] = bv[col : col + 64]
            bv_cat[0, 65 * h + 64] = 1.0
        in_maps.append(
            {
                "xT": np.ascontiguousarray(x[b].T).reshape(8, 128, N).astype(bf),
                "wq": np.ascontiguousarray(Wq[:, sl]).reshape(8, 128, 256).astype(bf),
                "wk": np.ascontiguousarray(Wk[:, sl]).reshape(8, 128, 256).astype(bf),
                "wv": wv_cat.reshape(8, 128, 260).astype(bf),
                "bq": np.ascontiguousarray(np.asarray(bq, f32)[sl].reshape(2, 128).T),
                "bk": np.ascontiguousarray(np.asarray(bk, f32)[sl].reshape(2, 128).T),
                "bvc": bv_cat.astype(bf),
                "wo": np.ascontiguousarray(
                    np.asarray(Wo, f32)[sl].reshape(2, 128, 1024).transpose(1, 0, 2)
                ).astype(bf),
                "mask": mask,
            }
        )
    return in_maps


def combine(results, bo):
    out = np.zeros((2, N, D), np.float32)
    for core in range(NC):
        out[core // 4] += results[core]["out"]
    out += np.asarray(bo, np.float32)[None, None, :]
    return out


def kernel(x, Wq, bq, Wk, bk, Wv, bv, Wo, bo):
    runner = _get_runner()
    in_maps = make_in_maps(x, Wq, bq, Wk, bk, Wv, bv, Wo, bo)
    dev_args = runner.prep(in_maps)
    results = runner.unpack(runner.run(dev_args))
    return combine(results, bo)


# revision 9
# speedup vs baseline: 294.6922x; 1.0011x over previous
"""Multi-head causal attention on 8 trn2 NeuronCores.

Sharding: data-parallel over batch (2) x tensor-parallel over heads (4 per
core, Megatron-style column-split QKV / row-split output projection).
Per-core partial outputs are summed on the host (+ output bias).
"""

import sys

sys.path.insert(0, "/opt/trn_rl_repo")

import ml_dtypes
import numpy as np

import concourse.bass as bass  # noqa: F401  (import keeps bass registered)
import concourse.tile as tile
from concourse import bacc, mybir

BF16 = mybir.dt.bfloat16
F32 = mybir.dt.float32
AF = mybir.ActivationFunctionType

N = 2048  # sequence length
D = 1024  # model dim
NC = 8  # cores


def build_nc(variant="full", loop=1):
    """Build the (SPMD) Bass program run identically on all 8 cores.

    variant: "full" | "nopb" (skip partition_broadcast, copy unnormalized ctx)
    loop: repeat the whole body N times inside the NEFF (timing harness).
    """
    nc = bacc.Bacc("TRN2", target_bir_lowering=False, debug=False, num_devices=NC)

    xT = nc.declare_dram_parameter("xT", [8, 128, N], BF16, isOutput=False)
    wq = nc.declare_dram_parameter("wq", [8, 128, 256], BF16, isOutput=False)
    wk = nc.declare_dram_parameter("wk", [8, 128, 256], BF16, isOutput=False)
    wv = nc.declare_dram_parameter("wv", [8, 128, 260], BF16, isOutput=False)
    bqp = nc.declare_dram_parameter("bq", [128, 2], F32, isOutput=False)
    bkp = nc.declare_dram_parameter("bk", [128, 2], F32, isOutput=False)
    bvcp = nc.declare_dram_parameter("bvc", [1, 260], BF16, isOutput=False)
    wo = nc.declare_dram_parameter("wo", [128, 2, 1024], BF16, isOutput=False)
    maskp = nc.declare_dram_parameter("mask", [128, 128], BF16, isOutput=False)
    outp = nc.declare_dram_parameter("out", [N, 1024], F32, isOutput=True)

    with tile.TileContext(nc) as tc:
        with tc.tile_pool(name="singles", bufs=1) as singles:
            xt_sb = singles.tile([128, 8, N], BF16)
            wq_sb = singles.tile([128, 8, 256], BF16)
            wk_sb = singles.tile([128, 8, 256], BF16)
            wv_sb = singles.tile([128, 8, 260], BF16)
            bq_sb = singles.tile([128, 2], F32)
            bk_sb = singles.tile([128, 2], F32)
            bvc_sb = singles.tile([1, 260], BF16)
            wo_sb = singles.tile([128, 2, 1024], BF16)
            mask_sb = singles.tile([128, 128], BF16)
            ones_sb = singles.tile([1, 128], BF16)
            qT_sb = singles.tile([128, 2, N], BF16)
            kT_sb = singles.tile([128, 2, N], BF16)
            vc_sb = singles.tile([128, 16, 260], BF16)
            ctxn_sb = singles.tile([128, 2, N], BF16)

            def _dma_in():
                nc.vector.memset(ones_sb[:, :], 1.0)
                # weights on the SWDGE path, activations on HWDGE — parallel
                # issue queues; one large strided DMA per tensor.
                nc.gpsimd.dma_start(
                    out=wq_sb[:, :, :], in_=wq[:, :, :].rearrange("k p n -> p k n")
                )
                nc.gpsimd.dma_start(
                    out=wk_sb[:, :, :], in_=wk[:, :, :].rearrange("k p n -> p k n")
                )
                nc.gpsimd.dma_start(
                    out=wv_sb[:, :, :], in_=wv[:, :, :].rearrange("k p n -> p k n")
                )
                nc.gpsimd.dma_start(out=bq_sb[:, :], in_=bqp[:, :])
                nc.gpsimd.dma_start(out=bk_sb[:, :], in_=bkp[:, :])
                nc.gpsimd.dma_start(out=bvc_sb[:, :], in_=bvcp[:, :])
                nc.gpsimd.dma_start(out=wo_sb[:, :, :], in_=wo[:, :, :])
                nc.gpsimd.dma_start(out=mask_sb[:, :], in_=maskp[:, :])
                nc.sync.dma_start(out=xt_sb[:, 0, :], in_=xT[0])
                nc.sync.dma_start(out=xt_sb[:, 1, :], in_=xT[1])
                for half in range(3):
                    k0 = 2 * half + 2
                    nc.sync.dma_start(
                        out=xt_sb[:, k0 : k0 + 2, :],
                        in_=xT[k0 : k0 + 2, :, :].rearrange("k p n -> p k n"),
                    )

            def _qk_proj(misc_ps, c):
                for w_sb, b_sb, o_sb in (
                    (wq_sb, bq_sb, qT_sb),
                    (wk_sb, bk_sb, kT_sb),
                ):
                    for I in range(4):
                        ps = misc_ps.tile([128, 1024], F32, tag="sc", name="qkps")
                        for kc in range(8):
                            nc.tensor.matmul(
                                ps[:, :512],
                                lhsT=w_sb[:, kc, 128 * c : 128 * (c + 1)],
                                rhs=xt_sb[:, kc, 512 * I : 512 * (I + 1)],
                                start=(kc == 0),
                                stop=(kc == 7),
                            )
                        nc.vector.tensor_scalar_add(
                            o_sb[:, c, 512 * I : 512 * (I + 1)],
                            ps[:, :512],
                            b_sb[:, c : c + 1],
                        )

            def _v_proj(misc_ps):
                for J in range(16):
                    ps = misc_ps.tile([128, 1024], F32, tag="sc", name="vps")
                    for kc in range(8):
                        nc.tensor.matmul(
                            ps[:, :260],
                            lhsT=xt_sb[:, kc, 128 * J : 128 * (J + 1)],
                            rhs=wv_sb[:, kc, :],
                            start=(kc == 0),
                            stop=False,
                        )
                    nc.tensor.matmul(
                        ps[:, :260],
                        lhsT=ones_sb[:, :],
                        rhs=bvc_sb[:, :],
                        start=False,
                        stop=True,
                    )
                    nc.vector.tensor_copy(out=vc_sb[:, J, :], in_=ps[:, :260])

            def _attn_head(misc_ps, ctxp, ptp, znp, h):
                c, po = h // 2, 64 * (h % 2)
                ctx_t = [
                    ctxp.tile([65, 512], F32, name=f"ctx{I}", tag=f"ctx{I}")
                    for I in range(4)
                ]
                for J in range(16):
                    L = N - 128 * J  # valid i starts at the diagonal
                    pt = ptp.tile([128, N], BF16, tag="pt", name="pt")
                    off = 0
                    while off < L:
                        w = min(1024, L - off)
                        ps = misc_ps.tile([128, 1024], F32, tag="sc", name="scps")
                        for s in range(0, w, 512):
                            sw = min(512, w - s)
                            gi = 128 * J + off + s
                            nc.tensor.matmul(
                                ps[:, s : s + sw],
                                lhsT=kT_sb[
                                    po : po + 64, c, 128 * J : 128 * (J + 1)
                                ],
                                rhs=qT_sb[po : po + 64, c, gi : gi + sw],
                                start=True,
                                stop=True,
                            )
                        nc.scalar.activation(
                            pt[:, off : off + w],
                            ps[:, :w],
                            AF.Exp,
                            scale=0.125,
                        )
                        off += w
                    # zero the j > i part of the diagonal block
                    nc.vector.tensor_mul(pt[:, :128], pt[:, :128], mask_sb[:, :])
                    for I in range(J // 4, 4):
                        gs = max(512 * I, 128 * J)
                        ge = 512 * (I + 1)
                        nc.tensor.matmul(
                            ctx_t[I][:, gs - 512 * I : ge - 512 * I],
                            lhsT=vc_sb[:, J, 65 * h : 65 * h + 65],
                            rhs=pt[:, gs - 128 * J : ge - 128 * J],
                            start=(J == 0),
                            stop=(J == 4 * I + 3),
                        )
                        if J != 4 * I + 3:
                            continue
                        # chunk I complete: normalize by 1/Z immediately
                        if variant == "nopb":
                            nc.vector.tensor_copy(
                                out=ctxn_sb[po : po + 64, c, 512 * I : 512 * (I + 1)],
                                in_=ctx_t[I][0:64, :],
                            )
                            continue
                        zr = znp.tile([1, 512], F32, tag="zr", name="zr")
                        nc.vector.reciprocal(zr[:, :], ctx_t[I][64:65, :])
                        zb = znp.tile([64, 512], F32, tag="zb", name="zb")
                        nc.gpsimd.partition_broadcast(zb[:, :], zr[:, :], channels=64)
                        nc.vector.tensor_mul(
                            ctxn_sb[po : po + 64, c, 512 * I : 512 * (I + 1)],
                            ctx_t[I][0:64, :],
                            zb[:, :],
                        )

            def _final(misc_ps, osb):
                for t in range(16):
                    for oc in range(2):
                        ps = misc_ps.tile([128, 1024], F32, tag="sc", name="fps")
                        for a in range(2):
                            nc.tensor.matmul(
                                ps[:, :512],
                                lhsT=ctxn_sb[:, a, 128 * t : 128 * (t + 1)],
                                rhs=wo_sb[:, a, 512 * oc : 512 * (oc + 1)],
                                start=(a == 0),
                                stop=(a == 1),
                            )
                        ot = osb.tile([128, 512], F32, tag="o", name="ot")
                        if (t + oc) % 2 == 0:
                            nc.vector.tensor_copy(out=ot[:, :], in_=ps[:, :512])
                        else:
                            nc.scalar.copy(out=ot[:, :], in_=ps[:, :512])
                        nc.sync.dma_start(
                            out=outp[
                                128 * t : 128 * (t + 1),
                                512 * oc : 512 * (oc + 1),
                            ],
                            in_=ot[:, :],
                        )

            def _iter():
                with tc.tile_pool(name="misc_ps", bufs=2, space="PSUM") as misc_ps, \
                     tc.tile_pool(name="ctx_ps", bufs=1, space="PSUM") as ctxp, \
                     tc.tile_pool(name="pt", bufs=3) as ptp, \
                     tc.tile_pool(name="zn", bufs=2) as znp, \
                     tc.tile_pool(name="osb", bufs=4) as osb:
                    _dma_in()
                    _qk_proj(misc_ps, 0)
                    _v_proj(misc_ps)
                    _attn_head(misc_ps, ctxp, ptp, znp, 0)
                    _qk_proj(misc_ps, 1)
                    _attn_head(misc_ps, ctxp, ptp, znp, 1)
                    _attn_head(misc_ps, ctxp, ptp, znp, 2)
                    _attn_head(misc_ps, ctxp, ptp, znp, 3)
                    _final(misc_ps, osb)

            if loop == 1:
                _iter()
            else:
                with tc.For_i(0, loop, 1):
                    _iter()

    nc.compile()
    return nc


class _Runner:
    """Jitted PJRT executor for the SPMD program (built once per process)."""

    def __init__(self, nc):
        import jax
        from jax.experimental.shard_map import shard_map
        from jax.sharding import Mesh, NamedSharding, PartitionSpec

        from concourse.bass2jax import (
            _bass_exec_p,
            install_neuronx_cc_hook,
            partition_id_tensor,
        )

        install_neuronx_cc_hook()
        self.nc = nc
        self.jax = jax

        in_names, out_names, out_avals = [], [], []
        partition_name = (
            nc.partition_id_tensor.name if nc.partition_id_tensor else None
        )
        for alloc in nc.m.functions[0].allocations:
            if not isinstance(alloc, mybir.MemoryLocationSet):
                continue
            name = alloc.memorylocations[0].name
            if alloc.kind == "ExternalInput":
                if name != partition_name:
                    in_names.append(name)
            elif alloc.kind == "ExternalOutput":
                out_names.append(name)
                out_avals.append(
                    jax.core.ShapedArray(
                        tuple(alloc.tensor_shape), mybir.dt.np(alloc.dtype)
                    )
                )
        self.in_names = list(in_names)
        self.out_names = out_names
        self.out_avals = out_avals
        n_params = len(in_names)
        n_outs = len(out_names)
        all_names = in_names + out_names
        if partition_name is not None:
            all_names = all_names + [partition_name]

        def _body(*args):
            operands = list(args)
            if partition_name is not None:
                operands.append(partition_id_tensor())
            return tuple(
                _bass_exec_p.bind(
                    *operands,
                    out_avals=tuple(out_avals),
                    in_names=tuple(all_names),
                    out_names=tuple(out_names),
                    lowering_input_output_aliases=(),
                    sim_require_finite=True,
                    sim_require_nnan=True,
                    nc=nc,
                )
            )

        devices = jax.devices()[:NC]
        self.mesh = Mesh(np.asarray(devices), ("core",))
        in_specs = (PartitionSpec("core"),) * (n_params + n_outs)
        out_specs = (PartitionSpec("core"),) * n_outs
        self.fn = jax.jit(
            shard_map(
                _body,
                mesh=self.mesh,
                in_specs=in_specs,
                out_specs=out_specs,
                check_rep=False,
            ),
            keep_unused=True,
        )
        self.sharding = NamedSharding(self.mesh, PartitionSpec("core"))

    def prep(self, in_maps):
        """Concatenate per-core inputs along axis 0 and device_put."""
        arrs = []
        for name in self.in_names:
            arrs.append(np.concatenate([m[name] for m in in_maps], axis=0))
        for av in self.out_avals:
            arrs.append(np.zeros((NC * av.shape[0], *av.shape[1:]), av.dtype))
        return [self.jax.device_put(a, self.sharding) for a in arrs]

    def run(self, dev_args):
        out = self.fn(*dev_args)
        self.jax.block_until_ready(out)
        return out

    def run_async(self, dev_args):
        return self.fn(*dev_args)

    def unpack(self, out):
        res = []
        for c in range(NC):
            res.append(
                {
                    name: np.asarray(out[i]).reshape(NC, *self.out_avals[i].shape)[c]
                    for i, name in enumerate(self.out_names)
                }
            )
        return res


_RUNNER = None


def _get_runner():
    global _RUNNER
    if _RUNNER is None:
        _RUNNER = _Runner(build_nc())
    return _RUNNER


def make_in_maps(x, Wq, bq, Wk, bk, Wv, bv, Wo, bo):
    bf = ml_dtypes.bfloat16
    f32 = np.float32
    x = np.asarray(x, f32)
    mask = np.ascontiguousarray(np.triu(np.ones((128, 128), f32))).astype(bf)
    in_maps = []
    for core in range(NC):
        b, g = core // 4, core % 4
        sl = slice(256 * g, 256 * (g + 1))
        wv_cat = np.zeros((D, 260), f32)
        bv_cat = np.zeros((1, 260), f32)
        for h in range(4):
            col = 256 * g + 64 * h
            wv_cat[:, 65 * h : 65 * h + 64] = Wv[:, col : col + 64]
            bv_cat[0, 65 * h : 65 * h + 64] = bv[col : col + 64]
            bv_cat[0, 65 * h + 64] = 1.0
        in_maps.append(
            {
                "xT": np.ascontiguousarray(x[b].T).reshape(8, 128, N).astype(bf),
                "wq": np.ascontiguousarray(Wq[:, sl]).reshape(8, 128, 256).astype(bf),
                "wk": np.ascontiguousarray(Wk[:, sl]).reshape(8, 128, 256).astype(bf),
                "wv": wv_cat.reshape(8, 128, 260).astype(bf),
                "bq": np.ascontiguousarray(np.asarray(bq, f32)[sl].reshape(2, 128).T),
                "bk": np.ascontiguousarray(np.asarray(bk, f32)[sl].reshape(2, 128).T),
                "bvc": bv_cat.astype(bf),
                "wo": np.ascontiguousarray(
                    np.asarray(Wo, f32)[sl].reshape(2, 128, 1024).transpose(1, 0, 2)
                ).astype(bf),
                "mask": mask,
            }
        )
    return in_maps


def combine(results, bo):
    out = np.zeros((2, N, D), np.float32)
    for core in range(NC):
        out[core // 4] += results[core]["out"]
    out += np.asarray(bo, np.float32)[None, None, :]
    return out


def kernel(x, Wq, bq, Wk, bk, Wv, bv, Wo, bo):
    runner = _get_runner()
    in_maps = make_in_maps(x, Wq, bq, Wk, bk, Wv, bv, Wo, bo)
    dev_args = runner.prep(in_maps)
    results = runner.unpack(runner.run(dev_args))
    return combine(results, bo)
